# revision 5
# baseline (speedup 1.0000x reference)
"""BlockCirculantConv on 8 Trainium2 NeuronCores.

The reference computes, per batch image b:
    xu = unfold(x[b])                       # (2304, 1024), f = c*9 + (di*3+dj)
    Y  = xu.flatten().reshape(1024, 2304)   # torch-faithful row-major reshape
    out_T = (Y @ W).T                       # W = expanded block-circulant (2304, 512)
    out[b] = out_T.reshape(512, 32, 32)
with W[q*64+s, p*64+t] = weight[p, q, (t-s) % 64]  (rfft product == circular conv).

Row n = 4c+j of Y is a contiguous 2304-chunk of channel c's 9 shifted images, so
the rhs S matrix S[k, j*256+c] (k = contraction row) is a gather of zero-padded
shifted images.  The gather is done ON HOST into the exact SBUF layout
sin[p, kt, j, c] (p = k%128, kt = k//128); weights pre-tiled to win[p, kt, m].

v3 is raw bass (no TileContext): hand-wired semaphores let the DMA triggers
issue at the very start of the kernel body and the matmul stream begin the
moment the first k-tile lands (running cold at 1.2 GHz while the HAM clock
gate warms), with a few const-fed dummy matmuls bridging the DMA latency.

Device kernel per core (data-parallel over batch, 1 image/core):
  - inputs fp16 (halves DMA bytes; fp32 PSUM accumulate; rel err ~4e-4)
  - input chunks split across both HWDGE rings (sync+scalar, each sustains
    only ~160-180 GB/s) sized so arrival stays ahead of the ~1.73us/k-tile
    consumption of the warm matmul stream
  - 8 PSUM banks accumulate out_T as 4 m-tiles x 2 column-halves over 18
    k-tiles; phase 1 (kt 0..13) round-robins all 8 psums, phase 2 (kt
    14..17) finishes one psum at a time so DVE drains + fp16 stores overlap
    the remaining matmuls; the very last psum finishes as two 256-col
    halves to shorten the final copy+store tail
  - output stored fp16 in (j*256+c) column order; host permutes columns
    back to n = 4c+j and casts to fp32.
"""

import sys

if "/opt/trn_rl_repo" not in sys.path:
    sys.path.insert(0, "/opt/trn_rl_repo")

import numpy as np

B, C, H, W_IMG = 8, 256, 32, 32
L = H * W_IMG               # 1024
BLK = 64
Q, P = 36, 8
K_FULL = Q * BLK            # 2304
M_OUT = P * BLK             # 512
KT = K_FULL // 128          # 18 k-tiles
N_CORES = 8
XT_ROWS = 1 + 34 * 32 + 1   # 1090 padded rows per dj copy

_CACHE = {}

SPLIT = 14                  # kt phase boundary: round-robin -> psum-major
N_WARM = 6                  # const-fed dummy matmuls bridging first DMA

# sin chunks on the sync ring (kt0 split by j-pair so the first matmul's
# rhs lands first), then the tail of sin rides the scalar ring behind win.
S_SYNC = [(0, 1, 0, 2), (0, 1, 2, 4), (1, 2, 0, 4), (2, 3, 0, 4),
          (3, 4, 0, 4), (4, 5, 0, 4), (5, 6, 0, 4), (6, 9, 0, 4),
          (9, 12, 0, 4)]
S_SCAL = [(12, 15, 0, 4), (15, 18, 0, 4)]
# win chunks on the scalar ring (kt0 split by m so lhsT of the first
# matmul lands first).
W_SCAL = [(0, 1, 0, 128), (0, 1, 128, 512), (1, 3, 0, 512), (3, 6, 0, 512),
          (6, 10, 0, 512), (10, 14, 0, 512), (14, 18, 0, 512)]


def _build_nc():
    import concourse.bacc as bacc
    import concourse.mybir as mybir

    dt = mybir.dt
    f16 = dt.float16
    f32 = dt.float32
    nc = bacc.Bacc("TRN2", target_bir_lowering=False, debug=False)

    sin = nc.dram_tensor("sin", [128, KT, 4, 256], f16, kind="ExternalInput").ap()
    win = nc.dram_tensor("win", [128, KT, M_OUT], f16, kind="ExternalInput").ap()
    out = nc.dram_tensor("out", [M_OUT, L], f16, kind="ExternalOutput").ap()

    sbig = nc.alloc_sbuf_tensor("sbig", [128, KT, 4, 256], f16).ap()
    wbig = nc.alloc_sbuf_tensor("wbig", [128, KT, M_OUT], f16).ap()
    wz = nc.alloc_sbuf_tensor("wz", [128, 256], f16).ap()
    ots = [nc.alloc_sbuf_tensor(f"ot{i}", [128, 512], f16).ap() for i in range(7)]
    oth = [nc.alloc_sbuf_tensor(f"oth{i}", [128, 256], f16).ap() for i in range(2)]
    psums = [nc.alloc_psum_tensor(f"ps{i}", [128, 512], f32).ap() for i in range(8)]

    s_sem = [nc.alloc_semaphore(f"s_sem{i}") for i in range(len(S_SYNC) + len(S_SCAL))]
    w_sem = [nc.alloc_semaphore(f"w_sem{i}") for i in range(len(W_SCAL))]
    wz_sem = nc.alloc_semaphore("wz_sem")
    pe_done = nc.alloc_semaphore("pe_done")
    cp_done = nc.alloc_semaphore("cp_done")
    out_done = nc.alloc_semaphore("out_done")

    # ---- input DMA triggers (first thing in the kernel body) ----
    for i, (a, b, ja, jb) in enumerate(S_SYNC):
        nc.sync.dma_start(sbig[:, a:b, ja:jb, :], sin[:, a:b, ja:jb, :]).then_inc(
            s_sem[i], 16
        )
    for i, (a, b, ma, mb) in enumerate(W_SCAL):
        nc.scalar.dma_start(wbig[:, a:b, ma:mb], win[:, a:b, ma:mb]).then_inc(
            w_sem[i], 16
        )
    for i, (a, b, ja, jb) in enumerate(S_SCAL):
        nc.scalar.dma_start(sbig[:, a:b, ja:jb, :], sin[:, a:b, ja:jb, :]).then_inc(
            s_sem[len(S_SYNC) + i], 16
        )

    nc.gpsimd.memset(wz, 0.0).then_inc(wz_sem)

    # sin chunk index per kt (kt0 handled specially), win chunk per kt
    s_chunk_of_kt = {}
    for i, (a, b, ja, jb) in enumerate(S_SYNC + S_SCAL):
        for kt in range(a, b):
            if kt > 0:
                s_chunk_of_kt[kt] = i
    w_chunk_of_kt = {}
    for i, (a, b, ma, mb) in enumerate(W_SCAL):
        for kt in range(a, b):
            if kt > 0:
                w_chunk_of_kt[kt] = i

    waited = set()

    def pe_wait(sem):
        if sem.name not in waited:
            nc.tensor.wait_ge(sem, 16)
            waited.add(sem.name)

    # ---- PE stream ----
    # warmup on zeros while the first chunks are in flight (HAM clock ramp)
    nc.tensor.wait_ge(wz_sem, 1)
    for _ in range(N_WARM):
        nc.tensor.matmul(psums[7][:, :256], wz[:, :128], wz, start=True, stop=True)

    def mm(kt, mt, nh, start, stop, n_half=None):
        lhsT = wbig[:, kt, mt * 128 : (mt + 1) * 128]
        if n_half is None:
            rhs = sbig[:, kt, nh * 2 : nh * 2 + 2, :]
            dst = psums[mt * 2 + nh]
        else:
            j = nh * 2 + n_half
            rhs = sbig[:, kt, j : j + 1, :]
            dst = psums[mt * 2 + nh][:, n_half * 256 : (n_half + 1) * 256]
        return nc.tensor.matmul(dst, lhsT, rhs, start=start, stop=stop)

    # kt0: weights-first order so the 32KB lhsT chunk gates only the 1st mm
    pe_wait(w_sem[0])
    pe_wait(s_sem[0])
    mm(0, 0, 0, True, False)
    pe_wait(w_sem[1])
    for mt in range(1, 4):
        mm(0, mt, 0, True, False)
    pe_wait(s_sem[1])
    for mt in range(4):
        mm(0, mt, 1, True, False)

    # phase 1: kt 1..SPLIT-1 round-robin over all 8 psums
    for kt in range(1, SPLIT):
        pe_wait(s_sem[s_chunk_of_kt[kt]])
        pe_wait(w_sem[w_chunk_of_kt[kt]])
        for mt in range(4):
            for nh in range(2):
                mm(kt, mt, nh, False, False)

    # phase 2: finish one psum at a time; last psum as two 256-col halves
    for mt in range(4):
        for nh in range(2):
            last = mt == 3 and nh == 1
            for kt in range(SPLIT, KT):
                pe_wait(s_sem[s_chunk_of_kt[kt]])
                pe_wait(w_sem[w_chunk_of_kt[kt]])
                if last and kt == KT - 1:
                    mm(kt, mt, nh, False, True, n_half=0).then_inc(pe_done)
                    mm(kt, mt, nh, False, True, n_half=1).then_inc(pe_done)
                elif kt == KT - 1:
                    mm(kt, mt, nh, False, True).then_inc(pe_done)
                else:
                    mm(kt, mt, nh, False, False)

    # ---- drains: DVE copy (fp32 psum -> fp16 sbuf), store on sync ring ----
    # drain order = psum finish order: 7 full psums then the two halves
    for i in range(7):
        nc.vector.wait_ge(pe_done, i + 1)
        nc.vector.tensor_copy(ots[i], psums[i]).then_inc(cp_done)
    for h in range(2):
        nc.vector.wait_ge(pe_done, 8 + h)
        nc.vector.tensor_copy(
            oth[h], psums[7][:, h * 256 : (h + 1) * 256]
        ).then_inc(cp_done)

    for i in range(7):
        mt, nh = divmod(i, 2)
        nc.sync.wait_ge(cp_done, i + 1)
        nc.sync.dma_start(
            out[mt * 128 : (mt + 1) * 128, nh * 512 : (nh + 1) * 512], ots[i]
        ).then_inc(out_done, 16)
    for h in range(2):
        nc.sync.wait_ge(cp_done, 8 + h)
        nc.sync.dma_start(
            out[384:512, 512 + h * 256 : 512 + (h + 1) * 256], oth[h]
        ).then_inc(out_done, 16)

    # make kernel completion wait for the last output bytes
    nc.sync.wait_ge(out_done, 16 * 9)

    nc.compile()
    return nc


def _host_prep(x, weight):
    x = np.ascontiguousarray(x, dtype=np.float32)
    weight = np.ascontiguousarray(weight, dtype=np.float32)

    # Expanded block-circulant matrix: W[q*64+s, p*64+t] = weight[p, q, (t-s)%64]
    idx = (np.arange(BLK)[None, :] - np.arange(BLK)[:, None]) % BLK   # (s, t)
    w4 = weight[:, :, idx]                                            # (p, q, s, t)
    wmat = w4.transpose(1, 2, 0, 3).reshape(K_FULL, M_OUT).astype(np.float16)
    win = np.ascontiguousarray(
        wmat.reshape(KT, 128, M_OUT).transpose(1, 0, 2)
    )                                                                 # (p, kt, m)

    # Shifted zero-padded transposed images: xt3[b, dj, 1+r*32+s, c]
    #   = x[b, c, r-1, s-1+dj] (zero outside the image)
    xp = x.transpose(0, 2, 3, 1).astype(np.float16)                   # (b, i, j, c)
    xt3 = np.zeros((B, 3, XT_ROWS, C), np.float16)
    v = xt3[:, :, 1 : 1 + 34 * 32, :].reshape(B, 3, 34, 32, C)
    v[:, 0, 1:33, 1:32] = xp[:, :, 0:31]
    v[:, 1, 1:33, 0:32] = xp
    v[:, 2, 1:33, 0:31] = xp[:, :, 1:32]

    # Gather into the device SBUF layout sin[b, p, kt, j, c]:
    #   k = kt*128+p, t = j*2304+k, dd = t//1024, l = t%1024,
    #   sin[...] = xt3[b, dd%3, 1 + (dd//3)*32 + l, c]
    t = np.arange(4)[None, :] * K_FULL + np.arange(K_FULL)[:, None]   # (k, j)
    dd, l = divmod(t, L)
    row = 1 + (dd // 3) * 32 + l
    vals = xt3[:, dd % 3, row, :]                                     # (b, k, j, c)
    sin = np.ascontiguousarray(
        vals.reshape(B, KT, 128, 4, C).transpose(0, 2, 1, 3, 4)
    )                                                                 # (b, p, kt, j, c)
    return sin, win


def _run(x, weight, trace=False, trace_kwargs=None):
    from concourse.bass_utils import run_bass_kernel_spmd

    if "nc" not in _CACHE:
        _CACHE["nc"] = _build_nc()
    nc = _CACHE["nc"]

    sin, win = _host_prep(x, weight)
    in_maps = [{"sin": sin[b], "win": win} for b in range(N_CORES)]
    res = run_bass_kernel_spmd(
        nc,
        in_maps,
        list(range(N_CORES)),
        trace=trace,
        **(trace_kwargs or {}),
    )
    out = np.stack([res.results[b]["out"] for b in range(N_CORES)])
    # device columns are (j*256 + c); output spatial index is n = 4c + j
    out = (
        out.reshape(B, M_OUT, 4, 256)
        .transpose(0, 1, 3, 2)
        .reshape(B, M_OUT, H, W_IMG)
        .astype(np.float32)
    )
    return np.ascontiguousarray(out), res


def kernel(x, weight):
    out, _ = _run(x, weight, trace=False)
    return out


# revision 8
# speedup vs baseline: 1.0293x; 1.0293x over previous
"""BlockCirculantConv on 8 Trainium2 NeuronCores.

The reference computes, per batch image b:
    xu = unfold(x[b])                       # (2304, 1024), f = c*9 + (di*3+dj)
    Y  = xu.flatten().reshape(1024, 2304)   # torch-faithful row-major reshape
    out_T = (Y @ W).T                       # W = expanded block-circulant (2304, 512)
    out[b] = out_T.reshape(512, 32, 32)
with W[q*64+s, p*64+t] = weight[p, q, (t-s) % 64]  (rfft product == circular conv).

Row n = 4c+j of Y is a contiguous 2304-chunk of channel c's 9 shifted images, so
the rhs S matrix S[k, j*256+c] (k = contraction row) is a gather of zero-padded
shifted images.  The gather is done ON HOST into the exact SBUF layout
sin[p, kt, j, c] (p = k%128, kt = k//128); weights pre-tiled to win[p, kt, m].

v3 is raw bass (no TileContext): hand-wired semaphores let the DMA triggers
issue at the very start of the kernel body and the matmul stream begin the
moment the first k-tile lands (running cold at 1.2 GHz while the HAM clock
gate warms), with a few const-fed dummy matmuls bridging the DMA latency.

Device kernel per core (data-parallel over batch, 1 image/core):
  - inputs fp16 (halves DMA bytes; fp32 PSUM accumulate; rel err ~4e-4)
  - input chunks split across both HWDGE rings (sync+scalar, each sustains
    only ~160-180 GB/s) sized so arrival stays ahead of the ~1.73us/k-tile
    consumption of the warm matmul stream
  - 8 PSUM banks accumulate out_T as 4 m-tiles x 2 column-halves over 18
    k-tiles; phase 1 (kt 0..13) round-robins all 8 psums, phase 2 (kt
    14..17) finishes one psum at a time so DVE drains + fp16 stores overlap
    the remaining matmuls; the very last psum finishes as two 256-col
    halves to shorten the final copy+store tail
  - output stored fp16 in (j*256+c) column order; host permutes columns
    back to n = 4c+j and casts to fp32.
"""

import sys

if "/opt/trn_rl_repo" not in sys.path:
    sys.path.insert(0, "/opt/trn_rl_repo")

import numpy as np

B, C, H, W_IMG = 8, 256, 32, 32
L = H * W_IMG               # 1024
BLK = 64
Q, P = 36, 8
K_FULL = Q * BLK            # 2304
M_OUT = P * BLK             # 512
KT = K_FULL // 128          # 18 k-tiles
N_CORES = 8
XT_ROWS = 1 + 34 * 32 + 1   # 1090 padded rows per dj copy

_CACHE = {}

SPLIT = 14                  # kt phase boundary: round-robin -> psum-major
N_WARM = 12                 # dummy matmuls: continuous PE activity until the
                            # first k-tile's DMA-completion sem is visible
                            # (~2.9us), so the HAM clock ramp never restarts

# sin chunks on the sync ring: kt0 split by j-pair so the first matmul's
# rhs lands first, then per-kt chunks (chunk completion gates consumption,
# so coarse chunks starve the stream); kt12+ rides the scalar ring.
S_SYNC = [(0, 1, 0, 2), (0, 1, 2, 4)] + [(k, k + 1, 0, 4) for k in range(1, 12)]
S_SCAL = [(k, k + 1, 0, 4) for k in range(12, 18)]
# win chunks on the scalar ring
W_SCAL = [(0, 1, 0, 512), (1, 3, 0, 512), (3, 6, 0, 512),
          (6, 10, 0, 512), (10, 14, 0, 512), (14, 18, 0, 512)]


def _build_nc():
    import concourse.bacc as bacc
    import concourse.mybir as mybir

    dt = mybir.dt
    f16 = dt.float16
    f32 = dt.float32
    nc = bacc.Bacc("TRN2", target_bir_lowering=False, debug=False)

    sin = nc.dram_tensor("sin", [128, KT, 4, 256], f16, kind="ExternalInput").ap()
    win = nc.dram_tensor("win", [128, KT, M_OUT], f16, kind="ExternalInput").ap()
    out = nc.dram_tensor("out", [M_OUT, L], f16, kind="ExternalOutput").ap()

    sbig = nc.alloc_sbuf_tensor("sbig", [128, KT, 4, 256], f16).ap()
    wbig = nc.alloc_sbuf_tensor("wbig", [128, KT, M_OUT], f16).ap()
    wz = nc.alloc_sbuf_tensor("wz", [128, 256], f16).ap()
    ots = [nc.alloc_sbuf_tensor(f"ot{i}", [128, 512], f16).ap() for i in range(7)]
    oth = [nc.alloc_sbuf_tensor(f"oth{i}", [128, 256], f16).ap() for i in range(2)]
    psums = [nc.alloc_psum_tensor(f"ps{i}", [128, 512], f32).ap() for i in range(8)]

    s_sem = [nc.alloc_semaphore(f"s_sem{i}") for i in range(len(S_SYNC) + len(S_SCAL))]
    w_sem = [nc.alloc_semaphore(f"w_sem{i}") for i in range(len(W_SCAL))]
    wz_sem = nc.alloc_semaphore("wz_sem")
    pe_done = nc.alloc_semaphore("pe_done")
    cp_done = nc.alloc_semaphore("cp_done")
    out_done = nc.alloc_semaphore("out_done")

    # ---- input DMA triggers (first thing in the kernel body) ----
    for i, (a, b, ja, jb) in enumerate(S_SYNC):
        nc.sync.dma_start(sbig[:, a:b, ja:jb, :], sin[:, a:b, ja:jb, :]).then_inc(
            s_sem[i], 16
        )
    for i, (a, b, ma, mb) in enumerate(W_SCAL):
        nc.scalar.dma_start(wbig[:, a:b, ma:mb], win[:, a:b, ma:mb]).then_inc(
            w_sem[i], 16
        )
    for i, (a, b, ja, jb) in enumerate(S_SCAL):
        nc.scalar.dma_start(sbig[:, a:b, ja:jb, :], sin[:, a:b, ja:jb, :]).then_inc(
            s_sem[len(S_SYNC) + i], 16
        )

    nc.gpsimd.memset(wz, 0.0).then_inc(wz_sem)

    # sin chunk index per kt (kt0 handled specially), win chunk per kt
    s_chunk_of_kt = {}
    for i, (a, b, ja, jb) in enumerate(S_SYNC + S_SCAL):
        for kt in range(a, b):
            if kt > 0:
                s_chunk_of_kt[kt] = i
    w_chunk_of_kt = {}
    for i, (a, b, ma, mb) in enumerate(W_SCAL):
        for kt in range(a, b):
            if kt > 0:
                w_chunk_of_kt[kt] = i

    waited = set()

    def pe_wait(sem):
        if sem.name not in waited:
            nc.tensor.wait_ge(sem, 16)
            waited.add(sem.name)

    # ---- PE stream ----
    # warmup on zeros while the first chunks are in flight (HAM clock ramp)
    nc.tensor.wait_ge(wz_sem, 1)
    for _ in range(N_WARM):
        nc.tensor.matmul(psums[7][:, :256], wz[:, :128], wz, start=True, stop=True)

    def mm(kt, mt, nh, start, stop, n_half=None):
        lhsT = wbig[:, kt, mt * 128 : (mt + 1) * 128]
        if n_half is None:
            rhs = sbig[:, kt, nh * 2 : nh * 2 + 2, :]
            dst = psums[mt * 2 + nh]
        else:
            j = nh * 2 + n_half
            rhs = sbig[:, kt, j : j + 1, :]
            dst = psums[mt * 2 + nh][:, n_half * 256 : (n_half + 1) * 256]
        return nc.tensor.matmul(dst, lhsT, rhs, start=start, stop=stop)

    # kt0: the j01 half of sin lands first, so run nh=0 across all m-tiles,
    # then nh=1 once the j23 half arrives
    pe_wait(w_sem[0])
    pe_wait(s_sem[0])
    for mt in range(4):
        mm(0, mt, 0, True, False)
    pe_wait(s_sem[1])
    for mt in range(4):
        mm(0, mt, 1, True, False)

    # phase 1: kt 1..SPLIT-1 round-robin over all 8 psums
    for kt in range(1, SPLIT):
        pe_wait(s_sem[s_chunk_of_kt[kt]])
        pe_wait(w_sem[w_chunk_of_kt[kt]])
        for mt in range(4):
            for nh in range(2):
                mm(kt, mt, nh, False, False)

    # phase 2: finish one psum at a time
    for mt in range(4):
        for nh in range(2):
            for kt in range(SPLIT, KT):
                pe_wait(s_sem[s_chunk_of_kt[kt]])
                pe_wait(w_sem[w_chunk_of_kt[kt]])
                if kt == KT - 1:
                    mm(kt, mt, nh, False, True).then_inc(pe_done)
                else:
                    mm(kt, mt, nh, False, False)

    # ---- drains: DVE copy (fp32 psum -> fp16 sbuf), store on sync ring ----
    # the last psum drains as two 256-col halves so the final copy+store
    # tail after the last matmul is half as long
    for i in range(7):
        nc.vector.wait_ge(pe_done, i + 1)
        nc.vector.tensor_copy(ots[i], psums[i]).then_inc(cp_done)
    nc.vector.wait_ge(pe_done, 8)
    for h in range(2):
        nc.vector.tensor_copy(
            oth[h], psums[7][:, h * 256 : (h + 1) * 256]
        ).then_inc(cp_done)

    for i in range(7):
        mt, nh = divmod(i, 2)
        nc.sync.wait_ge(cp_done, i + 1)
        nc.sync.dma_start(
            out[mt * 128 : (mt + 1) * 128, nh * 512 : (nh + 1) * 512], ots[i]
        ).then_inc(out_done, 16)
    for h in range(2):
        nc.sync.wait_ge(cp_done, 8 + h)
        nc.sync.dma_start(
            out[384:512, 512 + h * 256 : 512 + (h + 1) * 256], oth[h]
        ).then_inc(out_done, 16)

    # make kernel completion wait for the last output bytes
    nc.sync.wait_ge(out_done, 16 * 9)

    nc.compile()
    return nc


def _host_prep(x, weight):
    x = np.ascontiguousarray(x, dtype=np.float32)
    weight = np.ascontiguousarray(weight, dtype=np.float32)

    # Expanded block-circulant matrix: W[q*64+s, p*64+t] = weight[p, q, (t-s)%64]
    idx = (np.arange(BLK)[None, :] - np.arange(BLK)[:, None]) % BLK   # (s, t)
    w4 = weight[:, :, idx]                                            # (p, q, s, t)
    wmat = w4.transpose(1, 2, 0, 3).reshape(K_FULL, M_OUT).astype(np.float16)
    win = np.ascontiguousarray(
        wmat.reshape(KT, 128, M_OUT).transpose(1, 0, 2)
    )                                                                 # (p, kt, m)

    # Shifted zero-padded transposed images: xt3[b, dj, 1+r*32+s, c]
    #   = x[b, c, r-1, s-1+dj] (zero outside the image)
    xp = x.transpose(0, 2, 3, 1).astype(np.float16)                   # (b, i, j, c)
    xt3 = np.zeros((B, 3, XT_ROWS, C), np.float16)
    v = xt3[:, :, 1 : 1 + 34 * 32, :].reshape(B, 3, 34, 32, C)
    v[:, 0, 1:33, 1:32] = xp[:, :, 0:31]
    v[:, 1, 1:33, 0:32] = xp
    v[:, 2, 1:33, 0:31] = xp[:, :, 1:32]

    # Gather into the device SBUF layout sin[b, p, kt, j, c]:
    #   k = kt*128+p, t = j*2304+k, dd = t//1024, l = t%1024,
    #   sin[...] = xt3[b, dd%3, 1 + (dd//3)*32 + l, c]
    t = np.arange(4)[None, :] * K_FULL + np.arange(K_FULL)[:, None]   # (k, j)
    dd, l = divmod(t, L)
    row = 1 + (dd // 3) * 32 + l
    vals = xt3[:, dd % 3, row, :]                                     # (b, k, j, c)
    sin = np.ascontiguousarray(
        vals.reshape(B, KT, 128, 4, C).transpose(0, 2, 1, 3, 4)
    )                                                                 # (b, p, kt, j, c)
    return sin, win


def _run(x, weight, trace=False, trace_kwargs=None):
    from concourse.bass_utils import run_bass_kernel_spmd

    if "nc" not in _CACHE:
        _CACHE["nc"] = _build_nc()
    nc = _CACHE["nc"]

    sin, win = _host_prep(x, weight)
    in_maps = [{"sin": sin[b], "win": win} for b in range(N_CORES)]
    res = run_bass_kernel_spmd(
        nc,
        in_maps,
        list(range(N_CORES)),
        trace=trace,
        **(trace_kwargs or {}),
    )
    out = np.stack([res.results[b]["out"] for b in range(N_CORES)])
    # device columns are (j*256 + c); output spatial index is n = 4c + j
    out = (
        out.reshape(B, M_OUT, 4, 256)
        .transpose(0, 1, 3, 2)
        .reshape(B, M_OUT, H, W_IMG)
        .astype(np.float32)
    )
    return np.ascontiguousarray(out), res


def kernel(x, weight):
    out, _ = _run(x, weight, trace=False)
    return out
